# revision 1
# baseline (speedup 1.0000x reference)
"""Multi-head attention (B=4, N=2048, H=1024, 16 heads) on 8 NeuronCores.

Sharding: core c -> (batch b = c//2, head-group g = c%2) with 8 heads per
group.  Each core computes QKV projection for its group, attention over its
8 heads, and a partial out-projection against its group's w_out columns.
The host sums the two partial products per batch and adds b_out.

All on-device layouts avoid transposes entirely:
  - host supplies x[b].T (plus a ones row so qkv biases ride the contraction)
  - QT/KT are produced directly in [head-dims, tokens] layout
  - scoresT = KT.T-slices @ QT gives P already transposed for the PV matmul
  - a ones column appended to V yields the softmax denominator in the same
    PSUM accumulation as PV (max-subtraction-free softmax: scores ~ N(0,1),
    safely inside fp32 exp range)
"""

import numpy as np

B, N, H, NH = 4, 2048, 1024, 16
HD = 64
G = 2            # head-groups = cores per batch
GH = NH // G     # heads per group
GF = GH * HD     # features per group (512)
NPAIR = GH // 2  # head pairs per group
HT = 9           # h-tiles incl. bias row
AUG = HT * 128   # 1152
NT = N // 128    # token tiles
VW = GH * 65     # v tile width incl. ones columns

DTYPE = "f32r"   # "f32r" | "bf16" compute dtype for matmul operands

_NC_CACHE = {}


class _Ctx:
    pass


def _make_ctx(nc, dtype, rep):
    import concourse.mybir as mybir

    c = _Ctx()
    c.nc = nc
    c.mybir = mybir
    c.CD = mybir.dt.float32r if dtype == "f32r" else mybir.dt.bfloat16
    c.F32 = mybir.dt.float32
    c.Exp = mybir.ActivationFunctionType.Exp
    c.R = f"r{rep}_"
    return c


def _phase1(c, tc, xt_d, wqkv_d, qkT, vt):
    """QKV projection: fills qkT (QT pairs 0-3, KT pairs 4-7) and vt."""
    nc, R, CD, F32 = c.nc, c.R, c.CD, c.F32
    with (
        tc.tile_pool(name=f"{R}wq_pool", bufs=1) as wq_pool,
        tc.tile_pool(name=f"{R}xt_pool", bufs=18) as xt_pool,
        tc.tile_pool(name=f"{R}p1_psum", bufs=4, space="PSUM") as p1_psum,
    ):
        wq = [
            wq_pool.tile([128, 2 * GF + VW], CD, name=f"{R}wq{i}") for i in range(HT)
        ]
        for qb in range(4):  # 512-token column blocks
            xt = [
                xt_pool.tile([128, 512], CD, tag="xt", name=f"{R}xt_{qb}_{i}")
                for i in range(HT)
            ]
            for i in range(HT):
                if qb == 0:
                    # interleave weight loads with the first activation block
                    # so the first accumulation chain starts ~1 MB in, not 8 MB
                    nc.sync.dma_start(wq[i][:], wqkv_d[i * 128 : (i + 1) * 128, :])
                nc.sync.dma_start(
                    xt[i][:],
                    xt_d[i * 128 : (i + 1) * 128, qb * 512 : (qb + 1) * 512],
                )
            # QT/KT row-tiles: out[feat_pair_dims, tokens]
            for rt in range(8):
                ps = p1_psum.tile([128, 512], F32, tag="p1ps", name=f"{R}p1q_{qb}_{rt}")
                for ht in range(HT):
                    nc.tensor.matmul(
                        ps[:],
                        wq[ht][:, rt * 128 : (rt + 1) * 128],
                        xt[ht][:],
                        start=(ht == 0),
                        stop=(ht == HT - 1),
                    )
                nc.vector.tensor_copy(qkT[rt][:, qb * 512 : (qb + 1) * 512], ps[:])
            # V token-tiles: out[tokens, vfeat interleaved with ones cols]
            for vtl in range(4):
                tt = qb * 4 + vtl
                ps = p1_psum.tile([128, VW], F32, tag="p1ps", name=f"{R}p1v_{qb}_{vtl}")
                for ht in range(HT):
                    nc.tensor.matmul(
                        ps[:, 0:512],
                        xt[ht][:, vtl * 128 : (vtl + 1) * 128],
                        wq[ht][:, 2 * GF : 2 * GF + 512],
                        start=(ht == 0),
                        stop=(ht == HT - 1),
                    )
                    nc.tensor.matmul(
                        ps[:, 512:VW],
                        xt[ht][:, vtl * 128 : (vtl + 1) * 128],
                        wq[ht][:, 2 * GF + 512 : 2 * GF + VW],
                        start=(ht == 0),
                        stop=(ht == HT - 1),
                    )
                nc.vector.tensor_copy(vt[tt][:], ps[:])


def _phase2(c, tc, qkT, vt, attnT, ones64):
    """Attention per head pair; writes normalized transposed output attnT."""
    nc, R, CD, F32, Exp = c.nc, c.R, c.CD, c.F32, c.Exp
    QB2 = 1024
    DEPTH = 2  # software-pipeline depth: PV of iter i emitted after exp(i+DEPTH)
    with (
        tc.tile_pool(name=f"{R}pt_pool", bufs=6) as pt_pool,
        tc.tile_pool(name=f"{R}norm_pool", bufs=2) as norm_pool,
        tc.tile_pool(name=f"{R}ps_pool", bufs=2, space="PSUM") as ps_pool,
        tc.tile_pool(name=f"{R}po_pool", bufs=2, space="PSUM") as po_pool,
    ):
        for qb in range(N // QB2):
            for p in range(NPAIR):
                po = [
                    po_pool.tile([65, QB2], F32, tag="po", name=f"{R}po_{p}_{qb}_{h}")
                    for h in range(2)
                ]

                def emit_pv(item):
                    pt, ikt, h = item
                    vslice = vt[ikt][:, (p * 2 + h) * 65 : (p * 2 + h + 1) * 65]
                    for hf in range(2):
                        nc.tensor.matmul(
                            po[h][:, hf * 512 : (hf + 1) * 512],
                            vslice,
                            pt[:, hf * 512 : (hf + 1) * 512],
                            start=(ikt == 0),
                            stop=(ikt == NT - 1),
                        )

                pending = []
                for ikt in range(NT):
                    for h in range(2):
                        ps = ps_pool.tile(
                            [128, QB2], F32, tag="ps", name=f"{R}ps_{p}_{qb}_{ikt}_{h}"
                        )
                        for hf in range(2):
                            nc.tensor.matmul(
                                ps[:, hf * 512 : (hf + 1) * 512],
                                qkT[NPAIR + p][
                                    h * 64 : (h + 1) * 64, ikt * 128 : (ikt + 1) * 128
                                ],
                                qkT[p][
                                    h * 64 : (h + 1) * 64,
                                    qb * QB2 + hf * 512 : qb * QB2 + (hf + 1) * 512,
                                ],
                                start=True,
                                stop=True,
                                tile_position=(h * 64, 0),
                            )
                        pt = pt_pool.tile(
                            [128, QB2], CD, tag="pt", name=f"{R}pt_{p}_{qb}_{ikt}_{h}"
                        )
                        nc.scalar.activation(
                            pt[:], ps[:], Exp, scale=float(HD) ** -0.5
                        )
                        pending.append((pt, ikt, h))
                        if len(pending) > DEPTH:
                            emit_pv(pending.pop(0))
                for item in pending:
                    emit_pv(item)
                for h in range(2):
                    # evacuate the accumulator to SBUF so its 2 PSUM banks
                    # free before the normalization chain completes
                    poc = norm_pool.tile(
                        [65, QB2], F32, tag="poc", name=f"{R}poc_{p}_{qb}_{h}"
                    )
                    nc.vector.tensor_copy(poc[:], po[h][:])
                    recip = norm_pool.tile(
                        [1, QB2], F32, tag="recip", name=f"{R}rc_{p}_{qb}_{h}"
                    )
                    nc.vector.reciprocal(recip[:], poc[64:65, :])
                    pb = ps_pool.tile(
                        [64, QB2], F32, tag="ps", name=f"{R}pb_{p}_{qb}_{h}"
                    )
                    for hf in range(2):
                        nc.tensor.matmul(
                            pb[:, hf * 512 : (hf + 1) * 512],
                            ones64[:],
                            recip[:, hf * 512 : (hf + 1) * 512],
                            start=True,
                            stop=True,
                        )
                    bcast = norm_pool.tile(
                        [64, QB2], F32, tag="bcast", name=f"{R}bc_{p}_{qb}_{h}"
                    )
                    nc.vector.tensor_copy(bcast[:], pb[:])
                    nc.vector.tensor_mul(
                        attnT[p][h * 64 : (h + 1) * 64, qb * QB2 : (qb + 1) * QB2],
                        poc[0:64, :],
                        bcast[:],
                    )


def _phase3(c, tc, attnT, wo_d, out_d):
    """Partial out-projection: out = attnT.T @ wo."""
    nc, R, CD, F32 = c.nc, c.R, c.CD, c.F32
    with (
        tc.tile_pool(name=f"{R}wo_pool", bufs=1) as wo_pool,
        tc.tile_pool(name=f"{R}out_pool", bufs=3) as out_pool,
        tc.tile_pool(name=f"{R}p3_psum", bufs=4, space="PSUM") as p3_psum,
    ):
        wo = [wo_pool.tile([128, H], CD, name=f"{R}wo{i}") for i in range(NPAIR)]
        for i in range(NPAIR):
            nc.sync.dma_start(wo[i][:], wo_d[i * 128 : (i + 1) * 128, :])
        for tt in range(NT):
            ob = out_pool.tile([128, H], F32, tag="ob", name=f"{R}ob{tt}")
            for nb in range(2):
                ps = p3_psum.tile([128, 512], F32, tag="p3", name=f"{R}p3_{tt}_{nb}")
                for jt in range(NPAIR):
                    nc.tensor.matmul(
                        ps[:],
                        attnT[jt][:, tt * 128 : (tt + 1) * 128],
                        wo[jt][:, nb * 512 : (nb + 1) * 512],
                        start=(jt == 0),
                        stop=(jt == NPAIR - 1),
                    )
                nc.vector.tensor_copy(ob[:, nb * 512 : (nb + 1) * 512], ps[:])
            nc.sync.dma_start(out_d[tt * 128 : (tt + 1) * 128, :], ob[:])


def _build_body(c, tc, xt_d, wqkv_d, wo_d, out_d, phases):
    nc, R, CD, F32 = c.nc, c.R, c.CD, c.F32
    with (
        tc.tile_pool(name=f"{R}qk_pool", bufs=1) as qk_pool,
        tc.tile_pool(name=f"{R}v_pool", bufs=1) as v_pool,
        tc.tile_pool(name=f"{R}const_pool", bufs=1) as const_pool,
    ):
        qkT = [qk_pool.tile([128, N], CD, name=f"{R}qkT{i}") for i in range(8)]
        vt = [v_pool.tile([128, VW], CD, name=f"{R}v{i}") for i in range(NT)]
        ones64 = const_pool.tile([1, 64], F32, name=f"{R}ones64")
        nc.vector.memset(ones64[:], 1.0)

        if 1 in phases:
            _phase1(c, tc, xt_d, wqkv_d, qkT, vt)
        with tc.tile_pool(name=f"{R}attnT_pool", bufs=1) as attnT_pool:
            attnT = [
                attnT_pool.tile([128, N], CD, name=f"{R}attnT{i}")
                for i in range(NPAIR)
            ]
            if 2 in phases:
                _phase2(c, tc, qkT, vt, attnT, ones64)
            if 3 in phases:
                _phase3(c, tc, attnT, wo_d, out_d)


def _build_nc(reps=1, dtype=None, phases=(1, 2, 3)):
    from concourse import bacc
    import concourse.mybir as mybir
    import concourse.tile as tile

    dtype = dtype or DTYPE
    CD = mybir.dt.float32r if dtype == "f32r" else mybir.dt.bfloat16
    F32 = mybir.dt.float32

    nc = bacc.Bacc("TRN2", target_bir_lowering=False)
    xt_d = nc.dram_tensor("xt", [AUG, N], CD, kind="ExternalInput")
    # columns: Q (GF) | K (GF) | V interleaved per head [64 weights | ones]
    wqkv_d = nc.dram_tensor("wqkv", [AUG, 2 * GF + VW], CD, kind="ExternalInput")
    wo_d = nc.dram_tensor("wo", [GF, H], CD, kind="ExternalInput")
    out_d = nc.dram_tensor("out", [N, H], F32, kind="ExternalOutput")

    with tile.TileContext(nc) as tc:
        for rep in range(reps):
            c = _make_ctx(nc, dtype, rep)
            _build_body(c, tc, xt_d, wqkv_d, wo_d, out_d, phases)
    nc.finalize()
    return nc


def _get_nc():
    key = ("nc", DTYPE)
    if key not in _NC_CACHE:
        _NC_CACHE[key] = _build_nc()
    return _NC_CACHE[key]


def _np_dtype():
    if DTYPE == "f32r":
        return np.float32
    import ml_dtypes

    return ml_dtypes.bfloat16


def _prep_inputs(x, w_qkv, b_qkv, w_out):
    """Build per-core host-side input maps."""
    nd = _np_dtype()
    x = np.asarray(x, dtype=np.float32)
    w_qkv = np.asarray(w_qkv, dtype=np.float32)
    b_qkv = np.asarray(b_qkv, dtype=np.float32)
    w_out = np.asarray(w_out, dtype=np.float32)

    wqkv_aug, wo_t = [], []
    for g in range(G):
        w = np.zeros((AUG, 2 * GF + VW), np.float32)
        for k in range(2):  # q, k blocks of w_qkv rows
            rows = slice(k * H + g * GF, k * H + (g + 1) * GF)
            w[:H, k * GF : (k + 1) * GF] = w_qkv[rows, :].T
            w[H, k * GF : (k + 1) * GF] = b_qkv[rows]
        for h in range(GH):  # v block, 65 cols per head
            rows = slice(2 * H + g * GF + h * HD, 2 * H + g * GF + (h + 1) * HD)
            col = 2 * GF + h * 65
            w[:H, col : col + HD] = w_qkv[rows, :].T
            w[H, col : col + HD] = b_qkv[rows]
            w[H, col + HD] = 1.0
        wqkv_aug.append(w.astype(nd))
        wo_t.append(
            np.ascontiguousarray(w_out[:, g * GF : (g + 1) * GF].T).astype(nd)
        )

    xts = []
    for b in range(B):
        xa = np.zeros((AUG, N), np.float32)
        xa[:H] = x[b].T
        xa[H] = 1.0
        xts.append(xa.astype(nd))

    in_maps = []
    for cc in range(B * G):
        b, g = divmod(cc, G)
        in_maps.append({"xt": xts[b], "wqkv": wqkv_aug[g], "wo": wo_t[g]})
    return in_maps


def run_sharded(x, w_qkv, b_qkv, w_out, b_out, trace=False):
    """Run the SPMD kernel; returns (out, BassKernelResults)."""
    from concourse.bass_utils import run_bass_kernel_spmd

    in_maps = _prep_inputs(x, w_qkv, b_qkv, w_out)
    nc = _get_nc()
    bkr = run_bass_kernel_spmd(nc, in_maps, list(range(B * G)), trace=trace)
    res = bkr.results
    b_out = np.asarray(b_out, dtype=np.float32)
    out = np.empty((B, N, H), np.float32)
    for b in range(B):
        out[b] = res[G * b]["out"] + res[G * b + 1]["out"] + b_out[None, :]
    return out, bkr


def kernel(x, w_qkv, b_qkv, w_out, b_out):
    out, _ = run_sharded(x, w_qkv, b_qkv, w_out, b_out)
    return out



# revision 37
# speedup vs baseline: 14.8565x; 14.8565x over previous
"""Multi-head attention (B=4, N=2048, H=1024, 16 heads) on 8 NeuronCores.

Sharding: core c -> (batch b = c//2, head-group g = c%2), 8 heads per group.
Each core: QKV projection for its group, attention over its 8 heads, partial
out-projection against its group's w_out columns; host sums the two partials
per batch and adds b_out.

v4: single interleaved instruction stream built around the softmax-exp
(Activation engine) critical path:
  - minimal prefix (KT pair 0 + QT(0,0)) so the first exp fires ~15us in;
  - remaining projection tiles (KT pairs 1-3, QT per (p,qb), V token tiles)
    are injected one per attention slot from a fill queue, ordered by
    need-by time;
  - P-stationary PV with full-bank PSUM accumulators (4 chains of 65 per
    bank: only chain 0 start=True bank-clears; others overwrite-where-unset);
    softmax denominator rides as the ones-column, normalization is a
    per-partition reciprocal + tensor_scalar on DVE;
  - attq [q,f] -> attnT [f,q] via PE transpose; transposes and the partial
    out-projection for q-block i are injected during block i+1, so only the
    last block's tail work follows the final exp.
"""

import numpy as np

B, N, H, NH = 4, 2048, 1024, 16
HD = 64
G = 2            # head-groups = cores per batch
GH = NH // G     # heads per group (8)
GF = GH * HD     # features per group (512)
NPAIR = GH // 2  # head pairs per group (4)
NT = N // 128    # token tiles (16)
QB = 4           # q blocks of 512
QS = 4           # q subtiles per block

_NC_CACHE = {}


class _Ctx:
    pass


def _make_ctx(nc, rep):
    import concourse.mybir as mybir

    c = _Ctx()
    c.nc = nc
    c.mybir = mybir
    c.CD = mybir.dt.bfloat16
    c.F32 = mybir.dt.float32
    c.Exp = mybir.ActivationFunctionType.Exp
    c.R = f"r{rep}_"
    return c


def _build_body(c, tc, ht_n, xt_d, wqkv_d, wo_d, ident_d, out_d):
    nc, R, CD, F32, Exp = c.nc, c.R, c.CD, c.F32, c.Exp
    DEPTH = 2

    with (
        tc.tile_pool(name=f"{R}in_pool", bufs=1) as in_pool,
        tc.tile_pool(name=f"{R}qk_pool", bufs=1) as qk_pool,
        tc.tile_pool(name=f"{R}v_pool", bufs=1) as v_pool,
        tc.tile_pool(name=f"{R}attq_pool", bufs=1) as attq_pool,
        tc.tile_pool(name=f"{R}attnT_pool", bufs=1) as attnT_pool,
        tc.tile_pool(name=f"{R}pt_pool", bufs=6) as pt_pool,
        tc.tile_pool(name=f"{R}acs_pool", bufs=3) as acs_pool,
        tc.tile_pool(name=f"{R}rc_pool", bufs=2) as rc_pool,
        tc.tile_pool(name=f"{R}ob_pool", bufs=3) as ob_pool,
        tc.tile_pool(name=f"{R}p1_psum", bufs=2, space="PSUM") as p1_psum,
        tc.tile_pool(name=f"{R}ps_pool", bufs=2, space="PSUM") as ps_pool,
        tc.tile_pool(name=f"{R}acc_pool", bufs=2, space="PSUM") as acc_pool,
    ):
        xt = in_pool.tile([128, ht_n, N], CD, name=f"{R}xt")
        wq = in_pool.tile([128, ht_n, 3 * GF], CD, name=f"{R}wq")
        wo = in_pool.tile([128, 4, H], CD, name=f"{R}wo")
        ident = in_pool.tile([128, 128], CD, name=f"{R}ident")
        # DMA transfers serialize on the shared DMA fabric, so land the
        # first-attention-block inputs first: xt q-block 0, K weights, Q
        # weights, the remaining xt blocks, then V weights and the rest.
        def _rr(dram_slice):
            return dram_slice.rearrange("(a p) n -> p a n", p=128)

        def _xt_dma(qb):
            nc.sync.dma_start(
                xt[:, :, qb * 512 : (qb + 1) * 512],
                _rr(xt_d[:, qb * 512 : (qb + 1) * 512]),
            )

        _xt_dma(0)
        nc.sync.dma_start(wq[:, :, GF : 2 * GF], _rr(wqkv_d[:, GF : 2 * GF]))
        nc.sync.dma_start(wq[:, :, 0:128], _rr(wqkv_d[:, 0:128]))
        _xt_dma(1)
        nc.sync.dma_start(
            wq[:, :, 2 * GF : 2 * GF + 256], _rr(wqkv_d[:, 2 * GF : 2 * GF + 256])
        )
        _xt_dma(2)
        _xt_dma(3)
        nc.sync.dma_start(
            wq[:, :, 2 * GF + 256 :], _rr(wqkv_d[:, 2 * GF + 256 :])
        )
        nc.sync.dma_start(wq[:, :, 128:GF], _rr(wqkv_d[:, 128:GF]))
        nc.sync.dma_start(ident[:], ident_d[:, :])
        nc.sync.dma_start(wo[:, :, :], _rr(wo_d[:, :]))

        qkT = [qk_pool.tile([128, N], CD, name=f"{R}qkT{i}") for i in range(8)]
        vt = [v_pool.tile([128, 8, 65], CD, name=f"{R}v{i}") for i in range(NT)]
        attq = [attq_pool.tile([128, GF], CD, name=f"{R}attq{i}") for i in range(NT)]
        attnT = [
            attnT_pool.tile([128, N], CD, name=f"{R}attnT{i}") for i in range(NPAIR)
        ]

        # ---- phase-1 tile emitters ----------------------------------------
        qk_ps = {}

        def qk_tile(row, col0, qb, part=None):
            """qkT[row][:, qb*512:...] <- wq cols [col0:col0+128].T @ xt.

            part=0/1 emits half the contraction chain (separate fill slots
            keep un-interruptible PE stretches under one exp period)."""
            lo = 0 if part in (None, 0) else ht_n // 2
            hi = ht_n if part in (None, 1) else ht_n // 2
            if (row, qb) not in qk_ps:
                qk_ps[(row, qb)] = p1_psum.tile(
                    [128, 512], F32, tag="p1ps", name=f"{R}p1_{row}_{qb}"
                )
            ps = qk_ps[(row, qb)]
            for ht in range(lo, hi):
                nc.tensor.matmul(
                    ps[:],
                    wq[:, ht, col0 : col0 + 128],
                    xt[:, ht, qb * 512 : (qb + 1) * 512],
                    start=(ht == 0),
                    stop=(ht == ht_n - 1),
                )
            if hi == ht_n:
                nc.vector.tensor_copy(qkT[row][:, qb * 512 : (qb + 1) * 512], ps[:])

        def v_tile(tt, p):
            """vt[tt] value columns for head pair p (wq V cols p*128..)."""
            ps = p1_psum.tile([128, 512], F32, tag="p1ps", name=f"{R}p1v_{tt}_{p}")
            for ht in range(ht_n):
                nc.tensor.matmul(
                    ps[:, 0:128],
                    xt[:, ht, tt * 128 : (tt + 1) * 128],
                    wq[:, ht, 2 * GF + p * 128 : 2 * GF + (p + 1) * 128],
                    start=(ht == 0),
                    stop=(ht == ht_n - 1),
                )
            nc.vector.tensor_copy(
                vt[tt][:, 2 * p : 2 * p + 2, 0:64],
                ps[:, 0:128].rearrange("p (c w) -> p c w", c=2),
            )

        def tp_tile(qb, p):
            """attq[qb block][:, pair p cols] -> attnT[p][:, qb*512...]."""
            tp = p1_psum.tile([128, 1024], CD, tag="p1ps", name=f"{R}tp{qb}_{p}")
            for j in range(QS):
                nc.tensor.matmul(
                    tp[:, j * 128 : (j + 1) * 128],
                    attq[qb * QS + j][:, p * 128 : (p + 1) * 128],
                    ident[:],
                    is_transpose=True,
                )
            nc.vector.tensor_copy(
                attnT[p][:, qb * 512 : (qb + 1) * 512], tp[:, 0:512]
            )

        obs = {}

        def p3_half(tt, nb):
            """out rows tt*128..., column half nb: attnT.T @ wo."""
            if tt not in obs:
                obs[tt] = ob_pool.tile([128, H], F32, tag="ob", name=f"{R}ob{tt}")
            ob = obs[tt]
            ps = p1_psum.tile([128, 512], F32, tag="p1ps", name=f"{R}p3_{tt}_{nb}")
            for jt in range(NPAIR):
                nc.tensor.matmul(
                    ps[:],
                    attnT[jt][:, tt * 128 : (tt + 1) * 128],
                    wo[:, jt, nb * 512 : (nb + 1) * 512],
                    start=(jt == 0),
                    stop=(jt == NPAIR - 1),
                )
            nc.vector.tensor_copy(ob[:, nb * 512 : (nb + 1) * 512], ps[:])
            if nb == 1:
                nc.sync.dma_start(out_d[tt * 128 : (tt + 1) * 128, :], ob[:])

        # ---- fill queue: (need_by_slot, thunk), drained when due ----------
        fill = []

        def drain_due(s):
            while fill and fill[0][0] <= s:
                fill.pop(0)[1]()

        # PE p-state warmup: dummy matmuls while the input DMAs land
        scratch = in_pool.tile([128, 512], CD, name=f"{R}scratch")
        nc.vector.memset(scratch[:], 0.0)
        for tt in range(NT):
            nc.vector.memset(vt[tt][:, :, 64:65], 1.0)
        wps = p1_psum.tile([128, 512], F32, tag="p1ps", name=f"{R}warm")
        for i in range(22):
            nc.tensor.matmul(wps[:], scratch[:, 0:128], scratch[:],
                             start=(i == 0), stop=(i == 21))

        # prefix: just enough for the first few scores of block (qb0, p0)
        qk_tile(4 + 0, GF + 0 * 128, 0)
        qk_tile(0, 0, 0)

        # need-by slots for everything else (slot = global exp index;
        # window p covers slots 64p..64p+63, block (p,qb) starts 64p+16qb)
        def add_qk(due, row, col0, qb):
            for part in (0, 1):
                fill.append(
                    (max(0, due + part),
                     lambda row=row, col0=col0, qb=qb, part=part: qk_tile(
                         row, col0, qb, part))
                )

        for qb in range(1, 4):
            add_qk(4 * qb - 3, 4 + 0, GF, qb)
        for p in range(NPAIR):
            for tt in range(NT):
                fill.append(
                    (max(0, 64 * p + tt - 4), lambda tt=tt, p=p: v_tile(tt, p))
                )
        for p in range(1, NPAIR):
            for qb in range(4):
                add_qk(64 * (p - 1) + 20 + 5 * qb, 4 + p, GF + p * 128, qb)
        for p in range(NPAIR):
            for qb in range(4):
                if p == 0 and qb == 0:
                    continue  # in prefix
                due = 64 * (p - 1) + 44 if qb == 0 else 64 * p + 16 * qb - 8
                add_qk(due, p, p * 128, qb)
        fill.sort(key=lambda it: it[0])

        # ---- attention blocks (pair-major) --------------------------------
        slot = 0
        for p in range(NPAIR):
            for qb in range(QB):
                acc = [
                    acc_pool.tile(
                        [128, 512], F32, tag="acc", name=f"{R}acc_{qb}_{p}_{h}"
                    )
                    for h in range(2)
                ]

                def emit_pv(item, acc=acc, p=p):
                    pt, ikt = item
                    for h in range(2):
                        vs = vt[ikt][:, p * 2 + h, :]
                        for qs in range(QS):
                            nc.tensor.matmul(
                                acc[h][:, qs * 65 : (qs + 1) * 65],
                                pt[:, h * 512 + qs * 128 : h * 512 + (qs + 1) * 128],
                                vs,
                                start=(ikt == 0 and qs == 0),
                                stop=(ikt == NT - 1),
                                skip_group_check=True,
                            )

                pending = []
                for ikt in range(NT):
                    ps = ps_pool.tile(
                        [128, 1024], F32, tag="ps", name=f"{R}ps_{qb}_{p}_{ikt}"
                    )
                    for h in range(2):
                        nc.tensor.matmul(
                            ps[:, h * 512 : (h + 1) * 512],
                            qkT[NPAIR + p][
                                h * 64 : (h + 1) * 64, ikt * 128 : (ikt + 1) * 128
                            ],
                            qkT[p][h * 64 : (h + 1) * 64, qb * 512 : (qb + 1) * 512],
                            start=True,
                            stop=True,
                            tile_position=(h * 64, 0),
                        )
                    pt = pt_pool.tile(
                        [128, 1024], CD, tag="pt", name=f"{R}pt_{qb}_{p}_{ikt}"
                    )
                    nc.scalar.activation(pt[:], ps[:], Exp, scale=float(HD) ** -0.5)
                    drain_due(slot)
                    slot += 1
                    pending.append((pt, ikt))
                    if len(pending) > DEPTH:
                        emit_pv(pending.pop(0))
                for item in pending:
                    emit_pv(item)

                # evacuate accumulators; normalize per-q on DVE
                acs = [
                    acs_pool.tile(
                        [128, 4, 65], F32, tag="acs", name=f"{R}acs_{qb}_{p}_{h}"
                    )
                    for h in range(2)
                ]
                rc = rc_pool.tile([128, 2, 4], F32, tag="rc", name=f"{R}rc_{qb}_{p}")
                for h in range(2):
                    nc.vector.tensor_copy(
                        acs[h][:],
                        acc[h][:, 0:260].rearrange("p (c w) -> p c w", c=4),
                    )
                    nc.vector.reciprocal(rc[:, h, :], acs[h][:, :, 64])
                for h in range(2):
                    for qs in range(QS):
                        nc.vector.tensor_scalar_mul(
                            attq[qb * QS + qs][
                                :, (p * 2 + h) * 64 : (p * 2 + h + 1) * 64
                            ],
                            acs[h][:, qs, 0:64],
                            rc[:, h, qs : qs + 1],
                        )

                # block (p, qb) fully normalized: its transpose can run now
                fill.append(
                    (64 * p + 16 * (qb + 1) + 2, lambda qb=qb, p=p: tp_tile(qb, p))
                )
                # after the LAST pair's transpose, this q-block's rows of the
                # out-projection are ready; spread its halves over what
                # follows
                if p == NPAIR - 1:
                    for qs in range(QS):
                        for nb in range(2):
                            fill.append(
                                (192 + 16 * (qb + 1) + 4 + 2 * (2 * qs + nb),
                                 lambda tt=qb * QS + qs, nb=nb: p3_half(tt, nb))
                            )
                fill.sort(key=lambda it: it[0])

        while fill:
            fill.pop(0)[1]()


def _build_nc(reps=1, ht_n=8):
    from concourse import bacc
    import concourse.mybir as mybir
    import concourse.tile as tile

    CD = mybir.dt.bfloat16
    F32 = mybir.dt.float32

    nc = bacc.Bacc("TRN2", target_bir_lowering=False)
    xt_d = nc.dram_tensor("xt", [ht_n * 128, N], CD, kind="ExternalInput")
    # columns: Q (GF) | K (GF) | V (GF); rows: x features (+ bias row if ht_n=9)
    wqkv_d = nc.dram_tensor("wqkv", [ht_n * 128, 3 * GF], CD, kind="ExternalInput")
    wo_d = nc.dram_tensor("wo", [GF, H], CD, kind="ExternalInput")
    ident_d = nc.dram_tensor("ident", [128, 128], CD, kind="ExternalInput")
    out_d = nc.dram_tensor("out", [N, H], F32, kind="ExternalOutput")

    with tile.TileContext(nc) as tc:
        for rep in range(reps):
            c = _make_ctx(nc, rep)
            _build_body(c, tc, ht_n, xt_d, wqkv_d, wo_d, ident_d, out_d)
    nc.finalize()
    return nc


def _get_nc(ht_n):
    key = ("nc", ht_n)
    if key not in _NC_CACHE:
        _NC_CACHE[key] = _build_nc(ht_n=ht_n)
    return _NC_CACHE[key]


def _prep_inputs(x, w_qkv, b_qkv, w_out):
    """Build per-core host-side input maps."""
    import ml_dtypes

    nd = ml_dtypes.bfloat16
    x = np.asarray(x, dtype=np.float32)
    w_qkv = np.asarray(w_qkv, dtype=np.float32)
    b_qkv = np.asarray(b_qkv, dtype=np.float32)
    w_out = np.asarray(w_out, dtype=np.float32)

    ht_n = 9 if np.any(b_qkv) else 8
    aug = ht_n * 128

    wqkv_aug, wo_t = [], []
    for g in range(G):
        w = np.zeros((aug, 3 * GF), np.float32)
        for k in range(3):  # q, k, v blocks of w_qkv rows
            rows = slice(k * H + g * GF, k * H + (g + 1) * GF)
            w[:H, k * GF : (k + 1) * GF] = w_qkv[rows, :].T
            if ht_n == 9:
                w[H, k * GF : (k + 1) * GF] = b_qkv[rows]
        wqkv_aug.append(w.astype(nd))
        wo_t.append(
            np.ascontiguousarray(w_out[:, g * GF : (g + 1) * GF].T).astype(nd)
        )

    xts = []
    for b in range(B):
        xa = np.zeros((aug, N), np.float32)
        xa[:H] = x[b].T
        if ht_n == 9:
            xa[H] = 1.0
        xts.append(xa.astype(nd))

    ident = np.eye(128, dtype=np.float32).astype(nd)
    in_maps = []
    for cc in range(B * G):
        b, g = divmod(cc, G)
        in_maps.append(
            {"xt": xts[b], "wqkv": wqkv_aug[g], "wo": wo_t[g], "ident": ident}
        )
    return in_maps


def run_sharded(x, w_qkv, b_qkv, w_out, b_out, trace=False):
    """Run the SPMD kernel; returns (out, BassKernelResults)."""
    from concourse.bass_utils import run_bass_kernel_spmd

    in_maps = _prep_inputs(x, w_qkv, b_qkv, w_out)
    ht_n = in_maps[0]["xt"].shape[0] // 128
    nc = _get_nc(ht_n)
    bkr = run_bass_kernel_spmd(nc, in_maps, list(range(B * G)), trace=trace)
    res = bkr.results
    b_out = np.asarray(b_out, dtype=np.float32)
    out = np.empty((B, N, H), np.float32)
    for b in range(B):
        out[b] = res[G * b]["out"] + res[G * b + 1]["out"] + b_out[None, :]
    return out, bkr


def kernel(x, w_qkv, b_qkv, w_out, b_out):
    out, _ = run_sharded(x, w_qkv, b_qkv, w_out, b_out)
    return out


# revision 45
# speedup vs baseline: 15.2862x; 1.0289x over previous
"""Multi-head attention (B=4, N=2048, H=1024, 16 heads) on 8 NeuronCores.

Sharding: core c -> (batch b = c//2, head-group g = c%2), 8 heads per group.
Each core: QKV projection for its group, attention over its 8 heads, partial
out-projection against its group's w_out columns; host sums the two partials
per batch and adds b_out.

v4: single interleaved instruction stream built around the softmax-exp
(Activation engine) critical path:
  - minimal prefix (KT pair 0 + QT(0,0)) so the first exp fires ~15us in;
  - remaining projection tiles (KT pairs 1-3, QT per (p,qb), V token tiles)
    are injected one per attention slot from a fill queue, ordered by
    need-by time;
  - P-stationary PV with full-bank PSUM accumulators (4 chains of 65 per
    bank: only chain 0 start=True bank-clears; others overwrite-where-unset);
    softmax denominator rides as the ones-column, normalization is a
    per-partition reciprocal + tensor_scalar on DVE;
  - attq [q,f] -> attnT [f,q] via PE transpose; transposes and the partial
    out-projection for q-block i are injected during block i+1, so only the
    last block's tail work follows the final exp.
"""

import numpy as np

B, N, H, NH = 4, 2048, 1024, 16
HD = 64
G = 2            # head-groups = cores per batch
GH = NH // G     # heads per group (8)
GF = GH * HD     # features per group (512)
NPAIR = GH // 2  # head pairs per group (4)
NT = N // 128    # token tiles (16)
QB = 4           # q blocks of 512
QS = 4           # q subtiles per block

_NC_CACHE = {}


class _Ctx:
    pass


def _make_ctx(nc, rep):
    import concourse.mybir as mybir

    c = _Ctx()
    c.nc = nc
    c.mybir = mybir
    c.CD = mybir.dt.bfloat16
    c.F32 = mybir.dt.float32
    c.Exp = mybir.ActivationFunctionType.Exp
    c.R = f"r{rep}_"
    return c


def _build_body(c, tc, ht_n, xt_d, wqkv_d, wo_d, ident_d, out_d):
    nc, R, CD, F32, Exp = c.nc, c.R, c.CD, c.F32, c.Exp
    DEPTH = 2

    with (
        tc.tile_pool(name=f"{R}in_pool", bufs=1) as in_pool,
        tc.tile_pool(name=f"{R}qk_pool", bufs=1) as qk_pool,
        tc.tile_pool(name=f"{R}v_pool", bufs=1) as v_pool,
        tc.tile_pool(name=f"{R}attq_pool", bufs=1) as attq_pool,
        tc.tile_pool(name=f"{R}attnT_pool", bufs=1) as attnT_pool,
        tc.tile_pool(name=f"{R}pt_pool", bufs=6) as pt_pool,
        tc.tile_pool(name=f"{R}acs_pool", bufs=3) as acs_pool,
        tc.tile_pool(name=f"{R}rc_pool", bufs=2) as rc_pool,
        tc.tile_pool(name=f"{R}ob_pool", bufs=3) as ob_pool,
        tc.tile_pool(name=f"{R}p1_psum", bufs=2, space="PSUM") as p1_psum,
        tc.tile_pool(name=f"{R}ps_pool", bufs=2, space="PSUM") as ps_pool,
        tc.tile_pool(name=f"{R}acc_pool", bufs=2, space="PSUM") as acc_pool,
    ):
        xt = in_pool.tile([128, ht_n, N], CD, name=f"{R}xt")
        wq = in_pool.tile([128, ht_n, 3 * GF], CD, name=f"{R}wq")
        wo = in_pool.tile([128, 4, H], CD, name=f"{R}wo")
        ident = in_pool.tile([128, 128], CD, name=f"{R}ident")
        # DMA transfers serialize on the shared DMA fabric, so land the
        # first-attention-block inputs first: xt q-block 0, K weights, Q
        # weights, the remaining xt blocks, then V weights and the rest.
        def _rr(dram_slice):
            return dram_slice.rearrange("(a p) n -> p a n", p=128)

        def _xt_dma(qb):
            nc.sync.dma_start(
                xt[:, :, qb * 512 : (qb + 1) * 512],
                _rr(xt_d[:, qb * 512 : (qb + 1) * 512]),
            )

        _xt_dma(0)
        nc.sync.dma_start(
            wq[:, :, GF : GF + 128], _rr(wqkv_d[:, GF : GF + 128])
        )
        nc.sync.dma_start(wq[:, :, 0:128], _rr(wqkv_d[:, 0:128]))
        _xt_dma(1)
        nc.sync.dma_start(
            wq[:, :, 2 * GF : 2 * GF + 256], _rr(wqkv_d[:, 2 * GF : 2 * GF + 256])
        )
        _xt_dma(2)
        _xt_dma(3)
        nc.sync.dma_start(
            wq[:, :, GF + 128 : 2 * GF], _rr(wqkv_d[:, GF + 128 : 2 * GF])
        )
        nc.sync.dma_start(
            wq[:, :, 2 * GF + 256 :], _rr(wqkv_d[:, 2 * GF + 256 :])
        )
        nc.sync.dma_start(wq[:, :, 128:GF], _rr(wqkv_d[:, 128:GF]))
        nc.sync.dma_start(ident[:], ident_d[:, :])
        nc.sync.dma_start(wo[:, :, :], _rr(wo_d[:, :]))

        qkT = [qk_pool.tile([128, N], CD, name=f"{R}qkT{i}") for i in range(8)]
        vt = [v_pool.tile([128, 8, 65], CD, name=f"{R}v{i}") for i in range(NT)]
        attq = [attq_pool.tile([128, GF], CD, name=f"{R}attq{i}") for i in range(NT)]
        attnT = [
            attnT_pool.tile([128, N], CD, name=f"{R}attnT{i}") for i in range(NPAIR)
        ]

        # ---- phase-1 tile emitters ----------------------------------------
        qk_ps = {}

        def qk_tile(row, col0, qb, part=None):
            """qkT[row][:, qb*512:...] <- wq cols [col0:col0+128].T @ xt.

            part=0/1 emits half the contraction chain (separate fill slots
            keep un-interruptible PE stretches under one exp period)."""
            lo = 0 if part in (None, 0) else ht_n // 2
            hi = ht_n if part in (None, 1) else ht_n // 2
            if (row, qb) not in qk_ps:
                qk_ps[(row, qb)] = p1_psum.tile(
                    [128, 512], F32, tag="p1ps", name=f"{R}p1_{row}_{qb}"
                )
            ps = qk_ps[(row, qb)]
            for ht in range(lo, hi):
                nc.tensor.matmul(
                    ps[:],
                    wq[:, ht, col0 : col0 + 128],
                    xt[:, ht, qb * 512 : (qb + 1) * 512],
                    start=(ht == 0),
                    stop=(ht == ht_n - 1),
                )
            if hi == ht_n:
                nc.vector.tensor_copy(qkT[row][:, qb * 512 : (qb + 1) * 512], ps[:])

        def v_tile(tt, p):
            """vt[tt] value columns for head pair p (wq V cols p*128..)."""
            ps = p1_psum.tile([128, 512], F32, tag="p1ps", name=f"{R}p1v_{tt}_{p}")
            for ht in range(ht_n):
                nc.tensor.matmul(
                    ps[:, 0:128],
                    xt[:, ht, tt * 128 : (tt + 1) * 128],
                    wq[:, ht, 2 * GF + p * 128 : 2 * GF + (p + 1) * 128],
                    start=(ht == 0),
                    stop=(ht == ht_n - 1),
                )
            nc.vector.tensor_copy(
                vt[tt][:, 2 * p : 2 * p + 2, 0:64],
                ps[:, 0:128].rearrange("p (c w) -> p c w", c=2),
            )

        def tp_tile(qb, p):
            """attq[qb block][:, pair p cols] -> attnT[p][:, qb*512...]."""
            tp = p1_psum.tile([128, 1024], CD, tag="p1ps", name=f"{R}tp{qb}_{p}")
            for j in range(QS):
                nc.tensor.matmul(
                    tp[:, j * 128 : (j + 1) * 128],
                    attq[qb * QS + j][:, p * 128 : (p + 1) * 128],
                    ident[:],
                    is_transpose=True,
                )
            nc.vector.tensor_copy(
                attnT[p][:, qb * 512 : (qb + 1) * 512], tp[:, 0:512]
            )

        obs = {}

        def p3_half(tt, nb, tail=False):
            """out rows tt*128..., column half nb: attnT.T @ wo."""
            if tt not in obs:
                obs[tt] = ob_pool.tile([128, H], F32, tag="ob", name=f"{R}ob{tt}")
            ob = obs[tt]
            # in the tail the attention accumulator pool is idle; alternating
            # pools doubles the matmul/evacuate pipeline depth
            pool = acc_pool if tail and nb == 1 else p1_psum
            tag = "acc" if tail and nb == 1 else "p1ps"
            ps = pool.tile([128, 512], F32, tag=tag, name=f"{R}p3_{tt}_{nb}")
            for jt in range(NPAIR):
                nc.tensor.matmul(
                    ps[:],
                    attnT[jt][:, tt * 128 : (tt + 1) * 128],
                    wo[:, jt, nb * 512 : (nb + 1) * 512],
                    start=(jt == 0),
                    stop=(jt == NPAIR - 1),
                )
            nc.vector.tensor_copy(ob[:, nb * 512 : (nb + 1) * 512], ps[:])
            nc.sync.dma_start(
                out_d[tt * 128 : (tt + 1) * 128, nb * 512 : (nb + 1) * 512],
                ob[:, nb * 512 : (nb + 1) * 512],
            )

        # ---- fill queue: (need_by_slot, thunk), drained when due ----------
        fill = []

        def drain_due(s):
            while fill and fill[0][0] <= s:
                fill.pop(0)[1]()

        # PE p-state warmup: dummy matmuls while the input DMAs land
        scratch = in_pool.tile([128, 512], CD, name=f"{R}scratch")
        nc.vector.memset(scratch[:], 0.0)
        for tt in range(NT):
            nc.vector.memset(vt[tt][:, :, 64:65], 1.0)
        wps = p1_psum.tile([128, 512], F32, tag="p1ps", name=f"{R}warm")
        for i in range(16):
            nc.tensor.matmul(wps[:], scratch[:, 0:128], scratch[:],
                             start=(i == 0), stop=(i == 15))

        # prefix: just enough for the first few scores of block (qb0, p0)
        qk_tile(4 + 0, GF + 0 * 128, 0)
        qk_tile(0, 0, 0)

        # need-by slots for everything else (slot = global exp index;
        # window p covers slots 64p..64p+63, block (p,qb) starts 64p+16qb)
        def add_qk(due, row, col0, qb):
            for part in (0, 1):
                fill.append(
                    (max(0, due + part),
                     lambda row=row, col0=col0, qb=qb, part=part: qk_tile(
                         row, col0, qb, part))
                )

        for qb in range(1, 4):
            add_qk(4 * qb - 3, 4 + 0, GF, qb)
        for p in range(NPAIR):
            for tt in range(NT):
                due = tt - 4 if p == 0 else 64 * (p - 1) + 4 * tt + 2
                fill.append(
                    (max(0, due), lambda tt=tt, p=p: v_tile(tt, p))
                )
        for p in range(1, NPAIR):
            for qb in range(4):
                add_qk(64 * (p - 1) + 12 + 15 * qb, 4 + p, GF + p * 128, qb)
        for p in range(NPAIR):
            for qb in range(4):
                if p == 0 and qb == 0:
                    continue  # in prefix
                due = 64 * (p - 1) + 44 if qb == 0 else 64 * p + 16 * qb - 8
                add_qk(due, p, p * 128, qb)
        fill.sort(key=lambda it: it[0])

        # ---- attention blocks (pair-major) --------------------------------
        slot = 0
        for p in range(NPAIR):
            for qb in range(QB):
                acc = [
                    acc_pool.tile(
                        [128, 512], F32, tag="acc", name=f"{R}acc_{qb}_{p}_{h}"
                    )
                    for h in range(2)
                ]

                def emit_pv(item, acc=acc, p=p):
                    pt, ikt = item
                    for h in range(2):
                        vs = vt[ikt][:, p * 2 + h, :]
                        for qs in range(QS):
                            nc.tensor.matmul(
                                acc[h][:, qs * 65 : (qs + 1) * 65],
                                pt[:, h * 512 + qs * 128 : h * 512 + (qs + 1) * 128],
                                vs,
                                start=(ikt == 0 and qs == 0),
                                stop=(ikt == NT - 1),
                                skip_group_check=True,
                            )

                pending = []
                for ikt in range(NT):
                    ps = ps_pool.tile(
                        [128, 1024], F32, tag="ps", name=f"{R}ps_{qb}_{p}_{ikt}"
                    )
                    for h in range(2):
                        nc.tensor.matmul(
                            ps[:, h * 512 : (h + 1) * 512],
                            qkT[NPAIR + p][
                                h * 64 : (h + 1) * 64, ikt * 128 : (ikt + 1) * 128
                            ],
                            qkT[p][h * 64 : (h + 1) * 64, qb * 512 : (qb + 1) * 512],
                            start=True,
                            stop=True,
                            tile_position=(h * 64, 0),
                        )
                    pt = pt_pool.tile(
                        [128, 1024], CD, tag="pt", name=f"{R}pt_{qb}_{p}_{ikt}"
                    )
                    nc.scalar.activation(pt[:], ps[:], Exp, scale=float(HD) ** -0.5)
                    drain_due(slot)
                    slot += 1
                    pending.append((pt, ikt))
                    if len(pending) > DEPTH:
                        emit_pv(pending.pop(0))
                for item in pending:
                    emit_pv(item)

                # evacuate accumulators; normalize per-q on DVE
                acs = [
                    acs_pool.tile(
                        [128, 4, 65], F32, tag="acs", name=f"{R}acs_{qb}_{p}_{h}"
                    )
                    for h in range(2)
                ]
                rc = rc_pool.tile([128, 2, 4], F32, tag="rc", name=f"{R}rc_{qb}_{p}")
                for h in range(2):
                    nc.vector.tensor_copy(
                        acs[h][:],
                        acc[h][:, 0:260].rearrange("p (c w) -> p c w", c=4),
                    )
                    nc.vector.reciprocal(rc[:, h, :], acs[h][:, :, 64])
                for h in range(2):
                    for qs in range(QS):
                        nc.vector.tensor_scalar_mul(
                            attq[qb * QS + qs][
                                :, (p * 2 + h) * 64 : (p * 2 + h + 1) * 64
                            ],
                            acs[h][:, qs, 0:64],
                            rc[:, h, qs : qs + 1],
                        )

                # block (p, qb) fully normalized: its transpose can run now
                fill.append(
                    (64 * p + 16 * (qb + 1) + 2, lambda qb=qb, p=p: tp_tile(qb, p))
                )
                # after the LAST pair's transpose, this q-block's rows of the
                # out-projection are ready; spread its halves over what
                # follows
                if p == NPAIR - 1:
                    for qs in range(QS):
                        for nb in range(2):
                            fill.append(
                                (192 + 16 * (qb + 1) + 4 + 2 * (2 * qs + nb),
                                 lambda tt=qb * QS + qs, nb=nb, tail=(qb == 3):
                                     p3_half(tt, nb, tail))
                            )
                fill.sort(key=lambda it: it[0])

        while fill:
            fill.pop(0)[1]()


def _build_nc(reps=1, ht_n=8):
    from concourse import bacc
    import concourse.mybir as mybir
    import concourse.tile as tile

    CD = mybir.dt.bfloat16
    F32 = mybir.dt.float32

    nc = bacc.Bacc("TRN2", target_bir_lowering=False)
    xt_d = nc.dram_tensor("xt", [ht_n * 128, N], CD, kind="ExternalInput")
    # columns: Q (GF) | K (GF) | V (GF); rows: x features (+ bias row if ht_n=9)
    wqkv_d = nc.dram_tensor("wqkv", [ht_n * 128, 3 * GF], CD, kind="ExternalInput")
    wo_d = nc.dram_tensor("wo", [GF, H], CD, kind="ExternalInput")
    ident_d = nc.dram_tensor("ident", [128, 128], CD, kind="ExternalInput")
    out_d = nc.dram_tensor("out", [N, H], F32, kind="ExternalOutput")

    with tile.TileContext(nc) as tc:
        for rep in range(reps):
            c = _make_ctx(nc, rep)
            _build_body(c, tc, ht_n, xt_d, wqkv_d, wo_d, ident_d, out_d)
    nc.finalize()
    return nc


def _get_nc(ht_n):
    key = ("nc", ht_n)
    if key not in _NC_CACHE:
        _NC_CACHE[key] = _build_nc(ht_n=ht_n)
    return _NC_CACHE[key]


def _prep_inputs(x, w_qkv, b_qkv, w_out):
    """Build per-core host-side input maps."""
    import ml_dtypes

    nd = ml_dtypes.bfloat16
    x = np.asarray(x, dtype=np.float32)
    w_qkv = np.asarray(w_qkv, dtype=np.float32)
    b_qkv = np.asarray(b_qkv, dtype=np.float32)
    w_out = np.asarray(w_out, dtype=np.float32)

    ht_n = 9 if np.any(b_qkv) else 8
    aug = ht_n * 128

    wqkv_aug, wo_t = [], []
    for g in range(G):
        w = np.zeros((aug, 3 * GF), np.float32)
        for k in range(3):  # q, k, v blocks of w_qkv rows
            rows = slice(k * H + g * GF, k * H + (g + 1) * GF)
            w[:H, k * GF : (k + 1) * GF] = w_qkv[rows, :].T
            if ht_n == 9:
                w[H, k * GF : (k + 1) * GF] = b_qkv[rows]
        wqkv_aug.append(w.astype(nd))
        wo_t.append(
            np.ascontiguousarray(w_out[:, g * GF : (g + 1) * GF].T).astype(nd)
        )

    xts = []
    for b in range(B):
        xa = np.zeros((aug, N), np.float32)
        xa[:H] = x[b].T
        if ht_n == 9:
            xa[H] = 1.0
        xts.append(xa.astype(nd))

    ident = np.eye(128, dtype=np.float32).astype(nd)
    in_maps = []
    for cc in range(B * G):
        b, g = divmod(cc, G)
        in_maps.append(
            {"xt": xts[b], "wqkv": wqkv_aug[g], "wo": wo_t[g], "ident": ident}
        )
    return in_maps


def run_sharded(x, w_qkv, b_qkv, w_out, b_out, trace=False):
    """Run the SPMD kernel; returns (out, BassKernelResults)."""
    from concourse.bass_utils import run_bass_kernel_spmd

    in_maps = _prep_inputs(x, w_qkv, b_qkv, w_out)
    ht_n = in_maps[0]["xt"].shape[0] // 128
    nc = _get_nc(ht_n)
    bkr = run_bass_kernel_spmd(nc, in_maps, list(range(B * G)), trace=trace)
    res = bkr.results
    b_out = np.asarray(b_out, dtype=np.float32)
    out = np.empty((B, N, H), np.float32)
    for b in range(B):
        out[b] = res[G * b]["out"] + res[G * b + 1]["out"] + b_out[None, :]
    return out, bkr


def kernel(x, w_qkv, b_qkv, w_out, b_out):
    out, _ = run_sharded(x, w_qkv, b_qkv, w_out, b_out)
    return out
